# revision 19
# baseline (speedup 1.0000x reference)
"""Self-contained Trainium2 kernel for nn_Linear_14293651161742.

Computes y = act_dequant(act_quant(x)) @ (weight * expand(w_scale))^T which is
mathematically y[m,o] = sum_k x[m,k] * weight[o,k] * w_scale[o//128, k//128]
(the act_quant divide/multiply round-trip is an exact no-op up to fp32
rounding, far below the bf16 matmul noise floor).

Strategy: shard M across the 8 cores (column of the hint is worse: it
replicates the 128 MiB x per core; M-sharding needs only 96 MiB/core of HBM
traffic, leaving the kernel compute-bound at the bf16 PE roofline).

Schedule notes (v2):
- x is loaded f32 on the SAME HWDGE queues as the weight stream, interleaved
  per k-tile, so the FIFO order paces x vs w delivery in the exact ratio the
  round-0 matmuls consume them (the baseline's SWDGE x stream grabbed ~3.8x
  the DMA-engine share of the HWDGE w stream and starved round 0 of weights).
  The f32->bf16 cast runs on DVE, interleaved with the dequants in k order.
- PSUM evictions run on the (otherwise idle) Activation engine instead of
  DVE, so they can't be head-of-line blocked by prefetch dequants that are
  still waiting on weight DMA.
- Weights are staged from a chunk-contiguous host layout so every staging
  DMA is one linear 256 KiB region (2 KiB per-partition descriptors).

Host does layout prep only (transposes / scale replication); all arithmetic
(dequant, GEMM) runs on device.
"""

import sys

if "/opt/trn_rl_repo" not in sys.path:
    sys.path.insert(0, "/opt/trn_rl_repo")

import numpy as np

import concourse.bacc as bacc
import concourse.mybir as mybir
import concourse.tile as tile
from concourse import bass_utils

P = 128
N_CORES = 8

F32 = mybir.dt.float32
F32R = mybir.dt.float32r
BF16 = mybir.dt.bfloat16


def build_gemm_nc(M_loc: int, K: int, O: int):
    """Per-core program: yt[O, M_loc] = (wt * scale)^T-contracted with xt.

    Inputs (per core):
      xt  [K, M_loc] f32r : x slice, K-major (pre-transposed on host)
      wt  [OC, NG, P, WB, OCW] f32 : full weight, chunk-major staging blocks
                                     (wt[oc, g, p, i, c] = w^T[(g*WB+i)*P + p,
                                      oc*OCW + c])
      ws  [P, KT, OB] f32 : w_scale[ob, kb] replicated across partitions,
                            indexed [p, kb, ob]
    Output:
      yt  [O, M_loc] f32 : y^T slice (host transposes back)
    """
    KT = K // P            # k tiles
    OB = O // P            # 128-wide o tiles
    OCW = 256              # o-chunk width
    OC = O // OCW          # o chunks
    JT = OCW // P          # o tiles per chunk
    MCW = min(512, M_loc)  # matmul moving free dim
    MC = M_loc // MCW      # m chunks
    WB = 2                 # k-tiles per w staging DMA
    NG = KT // WB          # staging groups per chunk

    nc = bacc.Bacc("TRN2", target_bir_lowering=False, debug=False)
    xt = nc.dram_tensor("xt", [K, M_loc], F32, kind="ExternalInput")
    wt = nc.dram_tensor("wt", [OC, NG, P, WB, OCW], F32, kind="ExternalInput")
    ws = nc.dram_tensor("ws", [P, KT, OB], F32, kind="ExternalInput")
    yt = nc.dram_tensor("yt", [O, M_loc], F32, kind="ExternalOutput")

    xt_r = xt.ap().rearrange("(kt p) m -> p kt m", p=P)    # [P, KT, M_loc]
    wt_r = wt.ap().rearrange("oc g p i c -> oc g p i c")   # [OC, NG, P, WB, OCW]
    yt_r = yt.ap().rearrange("(ot p) m -> p ot m", p=P)    # [P, OB, M_loc]

    # Round schedule: round 0 covers two chunks (uses all 8 psum banks while
    # x streams in); later rounds one chunk each, psum double-buffered.
    rounds = [[0, 1]] + [[oc] for oc in range(2, OC)]

    with tile.TileContext(nc) as tc:
        with (
            tc.tile_pool(name="const", bufs=1) as const_pool,
            tc.tile_pool(name="xsb", bufs=1) as x_pool,
            tc.tile_pool(name="xstage", bufs=4) as xstage_pool,
            tc.tile_pool(name="wst_sp", bufs=6) as wstage_sp,
            tc.tile_pool(name="wst_sc", bufs=4) as wstage_sc,
            tc.tile_pool(name="wst_gp", bufs=8) as wstage_gp,
            tc.tile_pool(name="wbf", bufs=2) as wbf_pool,
            tc.tile_pool(name="yout", bufs=2) as y_pool,
            tc.tile_pool(name="psum", bufs=2, space="PSUM") as psum_pool,
        ):
            # scale tile rides the scalar queue, ahead of its wst/x stream
            ws_sb = const_pool.tile([P, KT, OB], F32)
            nc.scalar.dma_start(ws_sb[:], ws.ap())

            x_sb = [None] * KT
            w_chunks = {}  # oc -> list of KT bf16 [P, OCW] tiles

            def emit_x_load(kt, eng):
                xst = xstage_pool.tile([P, M_loc], F32, tag="xst", name="xst")
                eng.dma_start(xst[:], xt_r[:, kt, :])
                return xst

            def emit_x_cast(kt, xst, eng):
                xb = x_pool.tile([P, M_loc], BF16, tag=f"xb{kt}",
                                 name=f"xb{kt}")
                if eng is nc.vector:
                    nc.vector.tensor_copy(xb[:], xst[:])
                else:
                    nc.scalar.copy(xb[:], xst[:])
                x_sb[kt] = xb

            def emit_x_load_swdge(kt):
                # SWDGE casts f32->bf16 inline and adds DMA-engine read
                # bandwidth beyond the two paced HWDGE queues during the ramp
                xb = x_pool.tile([P, M_loc], BF16, tag=f"xb{kt}",
                                 name=f"xb{kt}")
                nc.gpsimd.dma_start(xb[:], xt_r[:, kt, :])
                x_sb[kt] = xb

            def emit_w_stage(oc, g, eng, pool):
                wst = pool.tile([P, WB, OCW], F32, tag="wst", name="wst")
                eng.dma_start(wst[:], wt_r[oc, g])
                return wst

            def emit_dequant(oc, g, wst):
                for i in range(WB):
                    kt = g * WB + i
                    wb = wbf_pool.tile([P, OCW], BF16, tag=f"wb{kt}",
                                       name=f"wb{kt}")
                    nc.vector.tensor_tensor(
                        wb.rearrange("p (g j) -> p g j", j=P),
                        wst[:, i].rearrange("p (g j) -> p g j", j=P),
                        ws_sb[:, kt, oc * JT:(oc + 1) * JT, None].to_broadcast(
                            [P, JT, P]
                        ),
                        mybir.AluOpType.mult,
                    )
                    w_chunks[oc][kt] = wb

            def emit_dequant_act(oc, g, wst):
                # ACT-engine dequant: the scale is constant along partitions
                # within one (kt, ob) block, so it rides the activation
                # per-partition `scale` operand.
                for i in range(WB):
                    kt = g * WB + i
                    wb = wbf_pool.tile([P, OCW], BF16, tag=f"wb{kt}",
                                       name=f"wb{kt}")
                    for j in range(JT):
                        nc.scalar.activation(
                            wb[:, j * P:(j + 1) * P],
                            wst[:, i, j * P:(j + 1) * P],
                            mybir.ActivationFunctionType.Copy,
                            scale=ws_sb[:, kt, oc * JT + j, None],
                        )
                    w_chunks[oc][kt] = wb

            # Prologue: chunks 0,1 + all of x, paced across the SP and ACT
            # HWDGE queues -- each queue's FIFO alternates w-stage and x so
            # delivery tracks round-0 consumption order.
            w_chunks[0] = [None] * KT
            w_chunks[1] = [None] * KT
            for g in range(NG):
                wst0 = emit_w_stage(0, g, nc.sync, wstage_sp)
                xst0 = emit_x_load(2 * g, nc.sync)
                wst1 = emit_w_stage(1, g, nc.scalar, wstage_sc)
                emit_x_load_swdge(2 * g + 1)
                emit_dequant(0, g, wst0)
                emit_dequant(1, g, wst1)
                emit_x_cast(2 * g, xst0, nc.vector)

            # Chunks 2,3 prefetch on the otherwise-idle SWDGE queue (separate
            # staging ring so it is not throttled by the prologue ring).
            for oc in (2, 3):
                w_chunks[oc] = [None] * KT
                for g in range(NG):
                    wst = emit_w_stage(oc, g, nc.gpsimd, wstage_gp)
                    emit_dequant(oc, g, wst)

            next_chunk = 4
            for rnd in rounds:
                # prefetch one upcoming chunk per processed chunk on the SP
                # queue (its prologue stream has drained by then)
                for _ in range(len(rnd)):
                    if next_chunk < OC:
                        w_chunks[next_chunk] = [None] * KT
                        for g in range(NG):
                            wst = emit_w_stage(next_chunk, g, nc.sync,
                                               wstage_sp)
                            emit_dequant(next_chunk, g, wst)
                        next_chunk += 1
                psums = {}
                for oc in rnd:
                    for j in range(JT):
                        for mc in range(MC):
                            psums[(oc, j, mc)] = psum_pool.tile(
                                [P, MCW], F32, tag=f"ps{j}_{mc}",
                                name=f"ps{j}_{mc}"
                            )
                for kt in range(KT):
                    for oc in rnd:
                        for j in range(JT):
                            lhsT = w_chunks[oc][kt][:, j * P:(j + 1) * P]
                            for mc in range(MC):
                                nc.tensor.matmul(
                                    psums[(oc, j, mc)][:],
                                    lhsT,
                                    x_sb[kt][:, mc * MCW:(mc + 1) * MCW],
                                    start=(kt == 0),
                                    stop=(kt == KT - 1),
                                )
                # evict on the ACT engine (idle otherwise): immune to DVE
                # dequant head-of-line blocking; store DMA follows in-order.
                for oc in rnd:
                    for mc in range(MC):
                        ysb = y_pool.tile([P, JT, MCW], F32, tag="ysb",
                                          name="ysb")
                        for j in range(JT):
                            nc.scalar.copy(ysb[:, j], psums[(oc, j, mc)][:])
                        nc.scalar.dma_start(
                            yt_r[:, oc * JT:(oc + 1) * JT,
                                 mc * MCW:(mc + 1) * MCW],
                            ysb[:],
                        )
                    del w_chunks[oc]
    nc.compile()
    return nc


_CACHED = {}


def _get_nc(M_loc, K, O):
    key = (M_loc, K, O)
    if key not in _CACHED:
        _CACHED[key] = build_gemm_nc(M_loc, K, O)
    return _CACHED[key]


def kernel(x: np.ndarray, weight: np.ndarray, w_scale: np.ndarray) -> np.ndarray:
    M, K = x.shape
    O = weight.shape[0]
    assert M % N_CORES == 0
    M_loc = M // N_CORES
    KT, OB = K // P, O // P
    OCW = 256
    OC = O // OCW
    WB = 2
    NG = KT // WB

    nc = _get_nc(M_loc, K, O)

    wt = np.ascontiguousarray(weight.T)                       # [K, O]
    # chunk-major staging blocks: [OC, NG, P, WB, OCW]
    wt5 = np.ascontiguousarray(
        wt.reshape(NG, WB, P, OC, OCW).transpose(3, 0, 2, 1, 4)
    )
    ws_rep = np.ascontiguousarray(
        np.broadcast_to(w_scale.T[None], (P, KT, OB))
    ).astype(np.float32)

    in_maps = []
    for c in range(N_CORES):
        xt_c = np.ascontiguousarray(x[c * M_loc:(c + 1) * M_loc, :].T)  # [K, M_loc]
        in_maps.append({"xt": xt_c, "wt": wt5, "ws": ws_rep})

    res = bass_utils.run_bass_kernel_spmd(
        nc, in_maps, core_ids=list(range(N_CORES))
    )
    return np.concatenate(
        [np.ascontiguousarray(res.results[c]["yt"].T) for c in range(N_CORES)],
        axis=0,
    )


# revision 24
# speedup vs baseline: 1.0178x; 1.0178x over previous
"""Self-contained Trainium2 kernel for nn_Linear_14293651161742.

Computes y = act_dequant(act_quant(x)) @ (weight * expand(w_scale))^T which is
mathematically y[m,o] = sum_k x[m,k] * weight[o,k] * w_scale[o//128, k//128]
(the act_quant divide/multiply round-trip is an exact no-op up to fp32
rounding, far below the bf16 matmul noise floor).

Strategy: shard M across the 8 cores (column of the hint is worse: it
replicates the 128 MiB x per core; M-sharding needs only 96 MiB/core of HBM
traffic, leaving the kernel compute-bound at the bf16 PE roofline).

Schedule notes (v2):
- x is loaded f32 on the SAME HWDGE queues as the weight stream, interleaved
  per k-tile, so the FIFO order paces x vs w delivery in the exact ratio the
  round-0 matmuls consume them (the baseline's SWDGE x stream grabbed ~3.8x
  the DMA-engine share of the HWDGE w stream and starved round 0 of weights).
  The f32->bf16 cast runs on DVE, interleaved with the dequants in k order.
- PSUM evictions run on the (otherwise idle) Activation engine instead of
  DVE, so they can't be head-of-line blocked by prefetch dequants that are
  still waiting on weight DMA.
- Weights are staged from a chunk-contiguous host layout so every staging
  DMA is one linear 256 KiB region (2 KiB per-partition descriptors).

Host does layout prep only (transposes / scale replication); all arithmetic
(dequant, GEMM) runs on device.
"""

import sys

if "/opt/trn_rl_repo" not in sys.path:
    sys.path.insert(0, "/opt/trn_rl_repo")

import numpy as np

import concourse.bacc as bacc
import concourse.mybir as mybir
import concourse.tile as tile
from concourse import bass_utils

P = 128
N_CORES = 8

F32 = mybir.dt.float32
F32R = mybir.dt.float32r
BF16 = mybir.dt.bfloat16


def build_gemm_nc(M_loc: int, K: int, O: int):
    """Per-core program: yt[O, M_loc] = (wt * scale)^T-contracted with xt.

    Inputs (per core):
      xt  [K, M_loc] f32r : x slice, K-major (pre-transposed on host)
      wt  [OC, NG, P, WB, OCW] f32 : full weight, chunk-major staging blocks
                                     (wt[oc, g, p, i, c] = w^T[(g*WB+i)*P + p,
                                      oc*OCW + c])
      ws  [P, KT, OB] f32 : w_scale[ob, kb] replicated across partitions,
                            indexed [p, kb, ob]
    Output:
      yt  [O, M_loc] f32 : y^T slice (host transposes back)
    """
    KT = K // P            # k tiles
    OB = O // P            # 128-wide o tiles
    OCW = 256              # o-chunk width
    OC = O // OCW          # o chunks
    JT = OCW // P          # o tiles per chunk
    MCW = min(512, M_loc)  # matmul moving free dim
    MC = M_loc // MCW      # m chunks
    WB = 2                 # k-tiles per w staging DMA
    NG = KT // WB          # staging groups per chunk

    nc = bacc.Bacc("TRN2", target_bir_lowering=False, debug=False)
    xt = nc.dram_tensor("xt", [K, M_loc], F32, kind="ExternalInput")
    wt = nc.dram_tensor("wt", [OC, NG, P, WB, OCW], F32, kind="ExternalInput")
    ws = nc.dram_tensor("ws", [P, KT, OB], F32, kind="ExternalInput")
    yt = nc.dram_tensor("yt", [O, M_loc], F32, kind="ExternalOutput")

    xt_r = xt.ap().rearrange("(kt p) m -> p kt m", p=P)    # [P, KT, M_loc]
    wt_r = wt.ap().rearrange("oc g p i c -> oc g p i c")   # [OC, NG, P, WB, OCW]
    yt_r = yt.ap().rearrange("(ot p) m -> p ot m", p=P)    # [P, OB, M_loc]

    # Round schedule: round 0 covers two chunks (uses all 8 psum banks while
    # x streams in); later rounds one chunk each, psum double-buffered.
    rounds = [[0, 1]] + [[oc] for oc in range(2, OC)]

    with tile.TileContext(nc) as tc:
        with (
            tc.tile_pool(name="const", bufs=1) as const_pool,
            tc.tile_pool(name="xsb", bufs=1) as x_pool,
            tc.tile_pool(name="xstage", bufs=8) as xstage_pool,
            tc.tile_pool(name="wst_sp", bufs=6) as wstage_sp,
            tc.tile_pool(name="wst_sc", bufs=4) as wstage_sc,
            tc.tile_pool(name="wst_gp", bufs=8) as wstage_gp,
            tc.tile_pool(name="wbf", bufs=2) as wbf_pool,
            tc.tile_pool(name="yout", bufs=2) as y_pool,
            tc.tile_pool(name="psum", bufs=2, space="PSUM") as psum_pool,
        ):
            # scale tile rides the scalar queue, ahead of its wst/x stream
            ws_sb = const_pool.tile([P, KT, OB], F32)
            nc.scalar.dma_start(ws_sb[:], ws.ap())

            x_sb = [None] * KT
            w_chunks = {}  # oc -> list of KT bf16 [P, OCW] tiles

            def emit_x_load(kt, mc, eng):
                # mc-granular x pieces: finer FIFO interleave primes the
                # pipeline faster and smooths the ramp-phase pacing
                xst = xstage_pool.tile([P, MCW], F32, tag="xst", name="xst")
                eng.dma_start(xst[:], xt_r[:, kt, mc * MCW:(mc + 1) * MCW])
                return xst

            def emit_x_cast(kt, mc, xst):
                xb = x_pool.tile([P, MCW], BF16, tag=f"xb{kt}_{mc}",
                                 name=f"xb{kt}_{mc}")
                nc.vector.tensor_copy(xb[:], xst[:])
                x_sb[kt][mc] = xb

            def emit_w_stage(oc, g, eng, pool):
                wst = pool.tile([P, WB, OCW], F32, tag="wst", name="wst")
                eng.dma_start(wst[:], wt_r[oc, g])
                return wst

            def emit_dequant(oc, g, wst):
                for i in range(WB):
                    kt = g * WB + i
                    wb = wbf_pool.tile([P, OCW], BF16, tag=f"wb{kt}",
                                       name=f"wb{kt}")
                    nc.vector.tensor_tensor(
                        wb.rearrange("p (g j) -> p g j", j=P),
                        wst[:, i].rearrange("p (g j) -> p g j", j=P),
                        ws_sb[:, kt, oc * JT:(oc + 1) * JT, None].to_broadcast(
                            [P, JT, P]
                        ),
                        mybir.AluOpType.mult,
                    )
                    w_chunks[oc][kt] = wb

            def emit_dequant_act(oc, g, wst):
                # ACT-engine dequant: the scale is constant along partitions
                # within one (kt, ob) block, so it rides the activation
                # per-partition `scale` operand.
                for i in range(WB):
                    kt = g * WB + i
                    wb = wbf_pool.tile([P, OCW], BF16, tag=f"wb{kt}",
                                       name=f"wb{kt}")
                    for j in range(JT):
                        nc.scalar.activation(
                            wb[:, j * P:(j + 1) * P],
                            wst[:, i, j * P:(j + 1) * P],
                            mybir.ActivationFunctionType.Copy,
                            scale=ws_sb[:, kt, oc * JT + j, None],
                        )
                    w_chunks[oc][kt] = wb

            # Prologue: chunks 0,1 + all of x, paced across the SP and ACT
            # HWDGE queues -- each queue's FIFO alternates w-stage and x so
            # delivery tracks round-0 consumption order.
            w_chunks[0] = [None] * KT
            w_chunks[1] = [None] * KT
            for kt in range(KT):
                x_sb[kt] = [None] * MC
            for g in range(NG):
                wst0 = emit_w_stage(0, g, nc.sync, wstage_sp)
                xsts0 = [emit_x_load(2 * g, mc, nc.sync) for mc in range(MC)]
                wst1 = emit_w_stage(1, g, nc.scalar, wstage_sc)
                xsts1 = [emit_x_load(2 * g + 1, mc, nc.scalar)
                         for mc in range(MC)]
                emit_dequant(0, g, wst0)
                emit_dequant(1, g, wst1)
                for mc in range(MC):
                    emit_x_cast(2 * g, mc, xsts0[mc])
                for mc in range(MC):
                    emit_x_cast(2 * g + 1, mc, xsts1[mc])

            # Chunks 2,3 prefetch on the otherwise-idle SWDGE queue (separate
            # staging ring so it is not throttled by the prologue ring).
            for oc in (2, 3):
                w_chunks[oc] = [None] * KT
                for g in range(NG):
                    wst = emit_w_stage(oc, g, nc.gpsimd, wstage_gp)
                    emit_dequant(oc, g, wst)

            next_chunk = 4
            for rnd in rounds:
                # prefetch one upcoming chunk per processed chunk on the SP
                # queue (its prologue stream has drained by then)
                for _ in range(len(rnd)):
                    if next_chunk < OC:
                        w_chunks[next_chunk] = [None] * KT
                        for g in range(NG):
                            wst = emit_w_stage(next_chunk, g, nc.sync,
                                               wstage_sp)
                            emit_dequant(next_chunk, g, wst)
                        next_chunk += 1
                psums = {}
                for oc in rnd:
                    for j in range(JT):
                        for mc in range(MC):
                            psums[(oc, j, mc)] = psum_pool.tile(
                                [P, MCW], F32, tag=f"ps{j}_{mc}",
                                name=f"ps{j}_{mc}"
                            )
                for kt in range(KT):
                    for oc in rnd:
                        for j in range(JT):
                            lhsT = w_chunks[oc][kt][:, j * P:(j + 1) * P]
                            for mc in range(MC):
                                nc.tensor.matmul(
                                    psums[(oc, j, mc)][:],
                                    lhsT,
                                    x_sb[kt][mc][:],
                                    start=(kt == 0),
                                    stop=(kt == KT - 1),
                                )
                # evict on the ACT engine (idle otherwise): immune to DVE
                # dequant head-of-line blocking; store DMA follows in-order.
                # Per-(j,mc) granularity so the final stores start sooner.
                for oc in rnd:
                    for j in range(JT):
                        for mc in range(MC):
                            ysb = y_pool.tile([P, MCW], F32, tag="ysb",
                                              name="ysb")
                            nc.scalar.copy(ysb[:], psums[(oc, j, mc)][:])
                            nc.scalar.dma_start(
                                yt_r[:, oc * JT + j,
                                     mc * MCW:(mc + 1) * MCW],
                                ysb[:],
                            )
                    del w_chunks[oc]
    nc.compile()
    return nc


_CACHED = {}


def _get_nc(M_loc, K, O):
    key = (M_loc, K, O)
    if key not in _CACHED:
        _CACHED[key] = build_gemm_nc(M_loc, K, O)
    return _CACHED[key]


def kernel(x: np.ndarray, weight: np.ndarray, w_scale: np.ndarray) -> np.ndarray:
    M, K = x.shape
    O = weight.shape[0]
    assert M % N_CORES == 0
    M_loc = M // N_CORES
    KT, OB = K // P, O // P
    OCW = 256
    OC = O // OCW
    WB = 2
    NG = KT // WB

    nc = _get_nc(M_loc, K, O)

    wt = np.ascontiguousarray(weight.T)                       # [K, O]
    # chunk-major staging blocks: [OC, NG, P, WB, OCW]
    wt5 = np.ascontiguousarray(
        wt.reshape(NG, WB, P, OC, OCW).transpose(3, 0, 2, 1, 4)
    )
    ws_rep = np.ascontiguousarray(
        np.broadcast_to(w_scale.T[None], (P, KT, OB))
    ).astype(np.float32)

    in_maps = []
    for c in range(N_CORES):
        xt_c = np.ascontiguousarray(x[c * M_loc:(c + 1) * M_loc, :].T)  # [K, M_loc]
        in_maps.append({"xt": xt_c, "wt": wt5, "ws": ws_rep})

    res = bass_utils.run_bass_kernel_spmd(
        nc, in_maps, core_ids=list(range(N_CORES))
    )
    return np.concatenate(
        [np.ascontiguousarray(res.results[c]["yt"].T) for c in range(N_CORES)],
        axis=0,
    )


# revision 30
# speedup vs baseline: 1.0335x; 1.0154x over previous
"""Self-contained Trainium2 kernel for nn_Linear_14293651161742.

Computes y = act_dequant(act_quant(x)) @ (weight * expand(w_scale))^T which is
mathematically y[m,o] = sum_k x[m,k] * weight[o,k] * w_scale[o//128, k//128]
(the act_quant divide/multiply round-trip is an exact no-op up to fp32
rounding, far below the bf16 matmul noise floor).

Strategy: shard M across the 8 cores (column of the hint is worse: it
replicates the 128 MiB x per core; M-sharding needs only 96 MiB/core of HBM
traffic, leaving the kernel compute-bound at the bf16 PE roofline).

Schedule notes (v2):
- x is loaded f32 on the SAME HWDGE queues as the weight stream, interleaved
  per k-tile, so the FIFO order paces x vs w delivery in the exact ratio the
  round-0 matmuls consume them (the baseline's SWDGE x stream grabbed ~3.8x
  the DMA-engine share of the HWDGE w stream and starved round 0 of weights).
  The f32->bf16 cast runs on DVE, interleaved with the dequants in k order.
- PSUM evictions run on the (otherwise idle) Activation engine instead of
  DVE, so they can't be head-of-line blocked by prefetch dequants that are
  still waiting on weight DMA.
- Weights are staged from a chunk-contiguous host layout so every staging
  DMA is one linear 256 KiB region (2 KiB per-partition descriptors).

Host does layout prep only (transposes / scale replication); all arithmetic
(dequant, GEMM) runs on device.
"""

import sys

if "/opt/trn_rl_repo" not in sys.path:
    sys.path.insert(0, "/opt/trn_rl_repo")

import numpy as np

import concourse.bacc as bacc
import concourse.mybir as mybir
import concourse.tile as tile
from concourse import bass_utils

P = 128
N_CORES = 8

F32 = mybir.dt.float32
F32R = mybir.dt.float32r
BF16 = mybir.dt.bfloat16


def build_gemm_nc(M_loc: int, K: int, O: int):
    """Per-core program: yt[O, M_loc] = (wt * scale)^T-contracted with xt.

    Inputs (per core):
      xt  [K, M_loc] f32r : x slice, K-major (pre-transposed on host)
      wt  [OC, NG, P, WB, OCW] f32 : full weight, chunk-major staging blocks
                                     (wt[oc, g, p, i, c] = w^T[(g*WB+i)*P + p,
                                      oc*OCW + c])
      ws  [P, KT, OB] f32 : w_scale[ob, kb] replicated across partitions,
                            indexed [p, kb, ob]
    Output:
      yt  [O, M_loc] f32 : y^T slice (host transposes back)
    """
    KT = K // P            # k tiles
    OB = O // P            # 128-wide o tiles
    OCW = 256              # o-chunk width
    OC = O // OCW          # o chunks
    JT = OCW // P          # o tiles per chunk
    MCW = min(512, M_loc)  # matmul moving free dim
    MC = M_loc // MCW      # m chunks
    WB = 2                 # k-tiles per w staging DMA
    NG = KT // WB          # staging groups per chunk

    nc = bacc.Bacc("TRN2", target_bir_lowering=False, debug=False)
    xt = nc.dram_tensor("xt", [K, M_loc], F32, kind="ExternalInput")
    wt = nc.dram_tensor("wt", [OC, NG, P, WB, OCW], F32, kind="ExternalInput")
    ws = nc.dram_tensor("ws", [P, KT, OB], F32, kind="ExternalInput")
    yt = nc.dram_tensor("yt", [O, M_loc], F32, kind="ExternalOutput")

    xt_r = xt.ap().rearrange("(kt p) m -> p kt m", p=P)    # [P, KT, M_loc]
    wt_r = wt.ap().rearrange("oc g p i c -> oc g p i c")   # [OC, NG, P, WB, OCW]
    yt_r = yt.ap().rearrange("(ot p) m -> p ot m", p=P)    # [P, OB, M_loc]

    # Round schedule: round 0 covers two chunks (uses all 8 psum banks while
    # x streams in); later rounds one chunk each, psum double-buffered.
    rounds = [[0, 1]] + [[oc] for oc in range(2, OC)]

    with tile.TileContext(nc) as tc:
        with (
            tc.tile_pool(name="const", bufs=1) as const_pool,
            tc.tile_pool(name="xsb", bufs=1) as x_pool,
            tc.tile_pool(name="xstage", bufs=4) as xstage_pool,
            tc.tile_pool(name="wst_sp", bufs=6) as wstage_sp,
            tc.tile_pool(name="wst_sc", bufs=4) as wstage_sc,
            tc.tile_pool(name="wst_gp", bufs=8) as wstage_gp,
            tc.tile_pool(name="wbf", bufs=2) as wbf_pool,
            tc.tile_pool(name="yout", bufs=2) as y_pool,
            tc.tile_pool(name="psum", bufs=2, space="PSUM") as psum_pool,
        ):
            # scale tile rides the scalar queue, ahead of its wst/x stream
            ws_sb = const_pool.tile([P, KT, OB], F32)
            nc.scalar.dma_start(ws_sb[:], ws.ap())

            x_sb = [None] * KT
            w_chunks = {}  # oc -> list of KT bf16 [P, OCW] tiles

            def emit_x_load(kt, eng):
                xst = xstage_pool.tile([P, M_loc], F32, tag="xst", name="xst")
                eng.dma_start(xst[:], xt_r[:, kt, :])
                return xst

            def emit_x_cast(kt, xst):
                xb = x_pool.tile([P, M_loc], BF16, tag=f"xb{kt}",
                                 name=f"xb{kt}")
                nc.vector.tensor_copy(xb[:], xst[:])
                x_sb[kt] = xb

            def emit_w_stage(oc, g, eng, pool):
                wst = pool.tile([P, WB, OCW], F32, tag="wst", name="wst")
                eng.dma_start(wst[:], wt_r[oc, g])
                return wst

            def emit_dequant(oc, g, wst):
                for i in range(WB):
                    kt = g * WB + i
                    wb = wbf_pool.tile([P, OCW], BF16, tag=f"wb{kt}",
                                       name=f"wb{kt}")
                    nc.vector.tensor_tensor(
                        wb.rearrange("p (g j) -> p g j", j=P),
                        wst[:, i].rearrange("p (g j) -> p g j", j=P),
                        ws_sb[:, kt, oc * JT:(oc + 1) * JT, None].to_broadcast(
                            [P, JT, P]
                        ),
                        mybir.AluOpType.mult,
                    )
                    w_chunks[oc][kt] = wb

            # Prologue: chunks 0,1 + all of x, paced across the SP and ACT
            # HWDGE queues -- each queue's FIFO alternates w-stage and x so
            # delivery tracks round-0 consumption order.
            w_chunks[0] = [None] * KT
            w_chunks[1] = [None] * KT
            for g in range(NG):
                wst0 = emit_w_stage(0, g, nc.sync, wstage_sp)
                xst0 = emit_x_load(2 * g, nc.sync)
                wst1 = emit_w_stage(1, g, nc.scalar, wstage_sc)
                xst1 = emit_x_load(2 * g + 1, nc.scalar)
                emit_dequant(0, g, wst0)
                emit_dequant(1, g, wst1)
                emit_x_cast(2 * g, xst0)
                emit_x_cast(2 * g + 1, xst1)

            # Chunks 2,3 prefetch on the otherwise-idle SWDGE queue (separate
            # staging ring so it is not throttled by the prologue ring).
            for oc in (2, 3):
                w_chunks[oc] = [None] * KT
                for g in range(NG):
                    wst = emit_w_stage(oc, g, nc.gpsimd, wstage_gp)
                    emit_dequant(oc, g, wst)

            next_chunk = 4
            for rnd in rounds:
                # prefetch one upcoming chunk per processed chunk on the SP
                # queue (its prologue stream has drained by then)
                for _ in range(len(rnd)):
                    if next_chunk < OC:
                        w_chunks[next_chunk] = [None] * KT
                        for g in range(NG):
                            wst = emit_w_stage(next_chunk, g, nc.sync,
                                               wstage_sp)
                            emit_dequant(next_chunk, g, wst)
                        next_chunk += 1
                psums = {}
                for oc in rnd:
                    for j in range(JT):
                        for mc in range(MC):
                            psums[(oc, j, mc)] = psum_pool.tile(
                                [P, MCW], F32, tag=f"ps{j}_{mc}",
                                name=f"ps{j}_{mc}"
                            )
                for kt in range(KT):
                    for oc in rnd:
                        for j in range(JT):
                            lhsT = w_chunks[oc][kt][:, j * P:(j + 1) * P]
                            for mc in range(MC):
                                nc.tensor.matmul(
                                    psums[(oc, j, mc)][:],
                                    lhsT,
                                    x_sb[kt][:, mc * MCW:(mc + 1) * MCW],
                                    start=(kt == 0),
                                    stop=(kt == KT - 1),
                                )
                # evict on the ACT engine (idle otherwise): immune to DVE
                # dequant head-of-line blocking; store DMA follows in-order.
                for oc in rnd:
                    for mc in range(MC):
                        ysb = y_pool.tile([P, JT, MCW], F32, tag="ysb",
                                          name="ysb")
                        for j in range(JT):
                            nc.scalar.copy(ysb[:, j], psums[(oc, j, mc)][:])
                        nc.scalar.dma_start(
                            yt_r[:, oc * JT:(oc + 1) * JT,
                                 mc * MCW:(mc + 1) * MCW],
                            ysb[:],
                        )
                    del w_chunks[oc]
    nc.compile()
    return nc


_CACHED = {}


def _get_nc(M_loc, K, O):
    key = (M_loc, K, O)
    if key not in _CACHED:
        _CACHED[key] = build_gemm_nc(M_loc, K, O)
    return _CACHED[key]


def kernel(x: np.ndarray, weight: np.ndarray, w_scale: np.ndarray) -> np.ndarray:
    M, K = x.shape
    O = weight.shape[0]
    assert M % N_CORES == 0
    M_loc = M // N_CORES
    KT, OB = K // P, O // P
    OCW = 256
    OC = O // OCW
    WB = 2
    NG = KT // WB

    nc = _get_nc(M_loc, K, O)

    wt = np.ascontiguousarray(weight.T)                       # [K, O]
    # chunk-major staging blocks: [OC, NG, P, WB, OCW]
    wt5 = np.ascontiguousarray(
        wt.reshape(NG, WB, P, OC, OCW).transpose(3, 0, 2, 1, 4)
    )
    ws_rep = np.ascontiguousarray(
        np.broadcast_to(w_scale.T[None], (P, KT, OB))
    ).astype(np.float32)

    in_maps = []
    for c in range(N_CORES):
        xt_c = np.ascontiguousarray(x[c * M_loc:(c + 1) * M_loc, :].T)  # [K, M_loc]
        in_maps.append({"xt": xt_c, "wt": wt5, "ws": ws_rep})

    res = bass_utils.run_bass_kernel_spmd(
        nc, in_maps, core_ids=list(range(N_CORES))
    )
    return np.concatenate(
        [np.ascontiguousarray(res.results[c]["yt"].T) for c in range(N_CORES)],
        axis=0,
    )
